# revision 18
# baseline (speedup 1.0000x reference)
"""Single-head causal attention (B=4, T=2048, C=1024) on 8 trn2 NeuronCores.

Sharding: 8 shards = (batch b in 0..3) x (query interleave h in 0..1); core h
takes interleaved 256-row query blocks {2*bg+h}, balancing the causal
triangle. One SPMD stream; per-core variation is data only (gathered q-rows
and three [128,128] mask tiles).

Math restructure vs the old baseline:
  - W_proj is folded into W_v host-side (Wt = w_proj @ w_v), deleting the
    output-projection phase entirely: y = (A @ (x Wt^T)) / rowsum + beff.
  - All matmuls run as fp8e4 DoubleRow (0.5 cycles/row, 256-deep contraction
    per instruction) with 3-term hi/lo error compensation: each operand v is
    split v = vh + vl (vh = e4m3(v), vl = e4m3(v - vh), both at natural
    scale) and products use vh*wh + vl*wh + vh*wl (the lo*lo term is ~eps^2
    and dropped). Effective precision ~bf16 at 0.75x the PE cost of bf16.
    Splits of x and the weights are free (host-side); k/q/V/A splits ride
    the existing PSUM-evacuation passes (Act: f32 scratch, Pool: hi cast,
    DVE: lo = scratch - hi).
  - Weights are shipped x32 (fp8-friendly range); the 1/32 un-scale rides
    the evacuation activations; 1/sqrt(C) rides the exp activation scale;
    exp carries a -ln(32) bias for fp8 headroom (cancels in softmax).
  - Everything stays in SBUF (fp8 halves footprints): no DRAM spill, no
    phase C reload, single attv accumulation chain over all 16 kv tiles.
"""

import sys

sys.path.insert(0, "/opt/trn_rl_repo")

import numpy as np
import ml_dtypes

import concourse.bass as bass
import concourse.tile as tile
from concourse import mybir
from concourse.vector_clock import ScopedClock
from bass_rust import AP as RAP

FP = mybir.dt.float32
BF = mybir.dt.bfloat16
F8 = mybir.dt.float8e4
AF = mybir.ActivationFunctionType
DR = mybir.MatmulPerfMode.DoubleRow
E4 = ml_dtypes.float8_e4m3

P = 128
C = 1024           # embed dim
NT = C // P        # 8 contraction tiles
T = 2048           # kv length per core
TK = T // P        # 16 kv tiles
H = 1024           # query cols per core
NEG = -1.0e9
ELN32 = -3.4657359  # -ln(32): exp headroom bias, cancels in softmax
S32 = 1.0 / 32.0

_MAX_WAITS = 1

# Interleaved-256 balanced causal structure (same tables as the baseline):
# query slots bg=0..3 hold global 256-row blocks g=2*bg+h. For kv tile S
# (0..15), valid query cols start at LO(S) = 512*(S//8) + LO128[S%8]*128,
# and MASKS[S%8] lists (query-128-block offset, mask tile) additions.
LO128 = [0, 0, 0, 1, 2, 2, 2, 3]
MASKS = [
    [(0, "m1d")],
    [(0, "m1f"), (1, "m1d")],
    [(0, "m2d"), (1, "m1f")],
    [(1, "m2d")],
    [(2, "m1d")],
    [(2, "m1f"), (3, "m1d")],
    [(2, "m2d"), (3, "m1f")],
    [(3, "m2d")],
]


def lo_of(S):
    return 512 * (S // 8) + LO128[S % 8] * P


class _TC(tile.TileContext):
    """TileContext whose tail drain puts its global-clock waits on a nop
    (walrus rejects multi-wait Drain); excess waits are split by
    _split_waits() afterwards."""

    def _drain_and_barrier(self, tick_clock, wait_clock):
        nop_inst = self.nc.sync.nop(nofuse=True, hint="pre_drain_waits")
        wait_clock.add_sem_waits(
            nop_inst.ins, ScopedClock({None: tick_clock.global_clock})
        )
        self.nc.sync.drain()
        self.nc.all_engine_barrier()
        assert self.sems is not None
        popped = self.nc._tile_sem_poison_stack.pop()
        assert popped is self._sem_poison
        self.nc.clear_and_free_semaphores(list(self.sems.allocated().values()))
        self.nc.all_engine_barrier()


def _split_waits(nc, max_waits=_MAX_WAITS):
    """Walrus rejects instructions carrying more than `max_waits` sync waits.
    Move excess waits onto injected nops placed immediately before the
    instruction on the same engine (identical semantics)."""
    import copy

    template = nc.sync.nop(nofuse=True, hint="waitsplit_template").ins
    counter = [0]

    def make_nop(engine, waits):
        nop = copy.deepcopy(template)
        counter[0] += 1
        nop.name = f"I-wsplit-{counter[0]}"
        nop.engine = engine
        nop.sync_info = mybir.SyncInfo(on_wait=list(waits), on_update=[])
        return nop

    f = nc.m.functions[0]
    for bb in f.blocks:
        insts = bb.instructions
        if not any(
            i.sync_info and i.sync_info.on_wait and len(i.sync_info.on_wait) > max_waits
            for i in insts
        ):
            continue
        newlist = []
        for inst in insts:
            si = inst.sync_info
            if si and si.on_wait and len(si.on_wait) > max_waits:
                if inst.name == template.name:
                    newlist.append(inst)
                    continue
                waits = list(si.on_wait)
                del si.on_wait[max_waits:]
                rest = waits[max_waits:]
                while rest:
                    newlist.append(make_nop(inst.engine, rest[:max_waits]))
                    rest = rest[max_waits:]
            newlist.append(inst)
        bb.instructions[:] = newlist


def _chunks(lo, hi, step=512):
    out = []
    while lo < hi:
        w = min(step, hi - lo)
        out.append((lo, lo + w))
        lo += w
    return out


def _pair(tl, off, stride, w):
    """[128, 2, w] AP over tile `tl` starting at column `off`, middle-dim
    stride `stride` (elements) — a DoubleRow operand covering two
    128-contraction slices."""
    a = tl[:]
    pstr, pcnt = a.ap[0]
    return RAP(a.tensor, a.offset + off, [[pstr, pcnt], [stride, 2], [1, w]])


def _build_nc():
    nc = bass.Bass("TRN2", target_bir_lowering=False, debug=False)

    # DRAM I/O.  x layouts: [p, ct*2048 + t] (hi | lo halves); xq likewise
    # with the core's gathered interleaved query rows.  Weights [p, hi|lo of
    # ot*1024 + ct*128 + o] (k/q, stationary layout) or [p, ct*1024 + ch]
    # (v-folded, moving layout), pre-scaled x32.
    xd = nc.dram_tensor("xd", [P, 2 * NT * T], F8, kind="ExternalInput").ap()
    xqd = nc.dram_tensor("xqd", [P, 2 * NT * H], F8, kind="ExternalInput").ap()
    wkd = nc.dram_tensor("wkd", [P, 2 * NT * C], F8, kind="ExternalInput").ap()
    wqd = nc.dram_tensor("wqd", [P, 2 * NT * C], F8, kind="ExternalInput").ap()
    wvd = nc.dram_tensor("wvd", [P, 2 * NT * C], F8, kind="ExternalInput").ap()
    bkd = nc.dram_tensor("bkd", [P, NT], FP, kind="ExternalInput").ap()
    bqd = nc.dram_tensor("bqd", [P, NT], FP, kind="ExternalInput").ap()
    bed = nc.dram_tensor("bed", [P, NT], FP, kind="ExternalInput").ap()
    onesd = nc.dram_tensor("onesd", [P, 2 * P], F8, kind="ExternalInput").ap()
    ones16d = nc.dram_tensor("ones16d", [P, 2 * P], F8, kind="ExternalInput").ap()
    m1dd = nc.dram_tensor("m1dd", [P, 2 * P], F8, kind="ExternalInput").ap()
    m1fd = nc.dram_tensor("m1fd", [P, 2 * P], F8, kind="ExternalInput").ap()
    m2dd = nc.dram_tensor("m2dd", [P, 2 * P], F8, kind="ExternalInput").ap()
    identd = nc.dram_tensor("identd", [P, 2 * P], F8, kind="ExternalInput").ap()
    ebd = nc.dram_tensor("ebd", [P, 2], FP, kind="ExternalInput").ap()
    # y out, bf16, tile-major: row block (ot*2 + chunk) holds [p, 512]
    yT = nc.dram_tensor("yT", [NT * 2 * P, 512], BF, kind="ExternalOutput").ap()

    with _TC(nc) as tc:
        with (
            tc.tile_pool(name="misc", bufs=1) as misc,
            tc.tile_pool(name="wpool", bufs=2) as wp,
            tc.tile_pool(name="wqp", bufs=4) as wqp,
            tc.tile_pool(name="kqv", bufs=1) as kqv,
            tc.tile_pool(name="scr", bufs=4) as scp,
            tc.tile_pool(name="yev", bufs=4) as yep,
            tc.tile_pool(name="psum", bufs=6, space="PSUM") as pp,
            tc.tile_pool(name="psum_rs", bufs=1, space="PSUM") as pp_rs,
        ):
            ones8 = misc.tile([P, 2 * P], F8, tag="ones")
            ones16 = misc.tile([P, 2 * P], F8, tag="ones16")
            m1d = misc.tile([P, 2 * P], F8, tag="m1d")
            m1f = misc.tile([P, 2 * P], F8, tag="m1f")
            m2d = misc.tile([P, 2 * P], F8, tag="m2d")
            ident8 = misc.tile([P, 2 * P], F8, tag="ident")
            bk_sb = misc.tile([P, NT], FP, tag="bk")
            bq_sb = misc.tile([P, NT], FP, tag="bq")
            be_sb = misc.tile([P, NT], FP, tag="be")
            dum = misc.tile([P, 640], F8, tag="dum")
            eb_sb = misc.tile([P, 2], FP, tag="eb")
            MT = {"m1d": m1d, "m1f": m1f, "m2d": m2d}

            kTh = kqv.tile([P, NT * T], F8, tag="kTh", name="kTh")
            kTl = kqv.tile([P, NT * T], F8, tag="kTl", name="kTl")
            qTh = kqv.tile([P, NT * H], F8, tag="qTh", name="qTh")
            qTl = kqv.tile([P, NT * H], F8, tag="qTl", name="qTl")
            vh = kqv.tile([P, TK * C], F8, tag="vh", name="vh")
            vl = kqv.tile([P, TK * C], F8, tag="vl", name="vl")
            v16 = kqv.tile([P, TK * C], F8, tag="v16", name="v16")

            def evac(ps, w, dsth, dstl, off, bias, scale, func=AF.Identity):
                """PSUM -> f32 scratch (Act) -> hi fp8 (Pool) -> lo fp8 (DVE)."""
                sc = scp.tile([P, 512], FP, tag="scr")
                nc.scalar.activation(sc[:, :w], ps[:, :w], func, bias=bias, scale=scale)
                nc.gpsimd.tensor_copy(dsth[:, off : off + w], sc[:, :w])
                nc.vector.tensor_sub(
                    dstl[:, off : off + w], sc[:, :w], dsth[:, off : off + w]
                )

            # 3-term DoubleRow accumulation helper.  terms = [(rhs_part_off,
            # lhs_part_off), ...] as (moving, stationary) hi/lo halves.
            TERMS = ((0, 0), (0, 1), (1, 0))

            # =========================================================
            # K projection: out [c(ot), kv] — lhsT = wk, rhs = x
            # =========================================================
            with tc.tile_pool(name="xp", bufs=1) as xp:
                xhl = xp.tile([P, 2 * NT * T], F8, tag="xhl", name="xhl")
                xq = xp.tile([P, 2 * NT * H], F8, tag="xq", name="xq")

                wk_h = wp.tile([P, NT * C], F8, tag="w", name="wk_h")
                wk_l = wp.tile([P, NT * C], F8, tag="w", name="wk_l")

                def wk_dma(ot):
                    nc.sync.dma_start(wk_h[:, ot * C : ot * C + C],
                                      wkd[:, ot * C : ot * C + C])
                    nc.sync.dma_start(wk_l[:, ot * C : ot * C + C],
                                      wkd[:, (NT + ot) * C : (NT + ot) * C + C])

                # critical-path DMA order: wk slice 0 (hi+lo), x chunk 0,
                # remaining wk slices, remaining x chunks, then the rest
                wk_dma(0)

                def xchunk(part, c0):
                    dst = RAP(
                        xhl[:].tensor,
                        xhl[:].offset + part * NT * T + c0,
                        [[xhl[:].ap[0][0], P], [T, NT], [1, 512]],
                    )
                    src = RAP(
                        xd.tensor,
                        xd.offset + part * NT * T + c0,
                        [[xd.ap[0][0], P], [T, NT], [1, 512]],
                    )
                    nc.sync.dma_start(dst, src)

                xchunk(0, 0)
                xchunk(1, 0)
                nc.sync.dma_start(bk_sb[:], bkd)
                for ot in range(1, NT):
                    wk_dma(ot)
                for c in range(1, 4):
                    xchunk(0, c * 512)
                    xchunk(1, c * 512)
                nc.sync.dma_start(xq[:], xqd)
                HC = 4 * C
                wq_h0 = wqp.tile([P, HC], F8, tag="wq", name="wq_h0")
                nc.sync.dma_start(wq_h0[:], wqd[:, :HC])
                wq_l0 = wqp.tile([P, HC], F8, tag="wq", name="wq_l0")
                nc.sync.dma_start(wq_l0[:], wqd[:, NT * C : NT * C + HC])
                wq_h1 = wqp.tile([P, HC], F8, tag="wq", name="wq_h1")
                nc.sync.dma_start(wq_h1[:], wqd[:, HC : 2 * HC])
                wq_l1 = wqp.tile([P, HC], F8, tag="wq", name="wq_l1")
                nc.sync.dma_start(wq_l1[:], wqd[:, NT * C + HC :])
                wqt = ((wq_h0, wq_l0), (wq_h1, wq_l1))
                nc.sync.dma_start(bq_sb[:], bqd)
                nc.sync.dma_start(eb_sb[:], ebd)

                sc_w = nc.named_scope("Wup")
                sc_w.__enter__()
                nc.gpsimd.memset(dum[:], 0.0)
                dps = pp.tile([P, 512], FP, tag="ps", name="ps_warm")
                for i in range(24):
                    nc.tensor.matmul(
                        dps[:],
                        lhsT=_pair(dum, 0, P, P),
                        rhs=_pair(dum, 0, P, 512),
                        start=True, stop=True, perf_mode=DR,
                    )
                sc_w.__exit__(None, None, None)

                sc_k = nc.named_scope("K")
                sc_k.__enter__()
                for cs, ce in _chunks(0, T):
                    for ot in range(NT):
                        w = ce - cs
                        ps = pp.tile([P, 512], FP, tag="ps", name=f"psk{ot}_{cs}")
                        n = 0
                        for rp, lp in TERMS:
                            for j in range(NT // 2):
                                nc.tensor.matmul(
                                    ps[:, :w],
                                    lhsT=_pair(
                                        wk_h if lp == 0 else wk_l,
                                        ot * C + j * 2 * P, P, P,
                                    ),
                                    rhs=_pair(
                                        xhl, rp * NT * T + j * 2 * T + cs, T, w
                                    ),
                                    start=(n == 0),
                                    stop=(n == 11),
                                    perf_mode=DR,
                                )
                                n += 1
                        evac(ps, w, kTh, kTl, ot * T + cs,
                             bk_sb[:, ot : ot + 1], S32)
                sc_k.__exit__(None, None, None)

                sc_q = nc.named_scope("Q")
                sc_q.__enter__()
                for ot in range(NT):
                    for cs, ce in _chunks(0, H):
                        w = ce - cs
                        ps = pp.tile([P, 512], FP, tag="ps", name=f"psq{ot}_{cs}")
                        n = 0
                        for rp, lp in TERMS:
                            for j in range(NT // 2):
                                nc.tensor.matmul(
                                    ps[:, :w],
                                    lhsT=_pair(
                                        wqt[ot // 4][lp],
                                        (ot % 4) * C + j * 2 * P, P, P,
                                    ),
                                    rhs=_pair(
                                        xq, rp * NT * H + j * 2 * H + cs, H, w
                                    ),
                                    start=(n == 0),
                                    stop=(n == 11),
                                    perf_mode=DR,
                                )
                                n += 1
                        evac(ps, w, qTh, qTl, ot * H + cs, bq_sb[:, ot : ot + 1], S32)
                sc_q.__exit__(None, None, None)

                wv_h = wp.tile([P, NT * C], F8, tag="w", name="wv_h")
                nc.sync.dma_start(wv_h[:], wvd[:, : NT * C])
                wv_l = wp.tile([P, NT * C], F8, tag="w", name="wv_l")
                nc.sync.dma_start(wv_l[:], wvd[:, NT * C :])
                nc.sync.dma_start(ones8[:], onesd)
                nc.sync.dma_start(ones16[:], ones16d)
                nc.sync.dma_start(m1d[:], m1dd)
                nc.sync.dma_start(m1f[:], m1fd)
                nc.sync.dma_start(m2d[:], m2dd)
                nc.sync.dma_start(ident8[:], identd)
                nc.sync.dma_start(be_sb[:], bed)

                sc_v = nc.named_scope("V")
                sc_v.__enter__()
                # folded-V projection: out [kv-rows(s), ch] — lhsT = x tile,
                # rhs = wv
                for s in range(TK):
                    for cs, ce in _chunks(0, C):
                        w = ce - cs
                        ps = pp.tile([P, 512], FP, tag="ps", name=f"psv{s}_{cs}")
                        n = 0
                        for rp, lp in TERMS:
                            for j in range(NT // 2):
                                nc.tensor.matmul(
                                    ps[:, :w],
                                    lhsT=_pair(
                                        xhl, lp * NT * T + j * 2 * T + s * P, T, P
                                    ),
                                    rhs=_pair(
                                        wv_h if rp == 0 else wv_l,
                                        j * 2 * C + cs, C, w,
                                    ),
                                    start=(n == 0),
                                    stop=(n == 11),
                                    perf_mode=DR,
                                )
                                n += 1
                        off = s * C + cs
                        sc = scp.tile([P, 512], FP, tag="scr")
                        nc.scalar.activation(
                            sc[:, :w], ps[:, :w], AF.Identity,
                            bias=eb_sb[:, 1:2], scale=S32,
                        )
                        nc.gpsimd.tensor_copy(vh[:, off : off + w], sc[:, :w])
                        nc.vector.tensor_sub(
                            vl[:, off : off + w], sc[:, :w], vh[:, off : off + w]
                        )
                        nc.vector.tensor_scalar_mul(
                            v16[:, off : off + w], sc[:, :w], 1.0 / 16.0
                        )
                sc_v.__exit__(None, None, None)

            # =========================================================
            # Attention: x pool freed, A tensors reuse its space
            # =========================================================
            with tc.tile_pool(name="ap", bufs=1) as apool:
                Ah = apool.tile([P, TK * H], F8, tag="Ah", name="Ah")
                rs_sb = apool.tile([P, H], FP, tag="rs", name="rs_sb")
                Al16 = apool.tile([P, TK * H], F8, tag="Al16", name="Al16")
                rs_ps = pp_rs.tile([P, H], FP, tag="rsps")

                # zero the pair-union gap regions (read by rowsum/attv,
                # never written by scores): tiles S=3,7,11,15
                for S in (3, 7, 11, 15):
                    g0 = lo_of(S - 1)
                    g1 = lo_of(S)
                    nc.gpsimd.memset(Ah[:, S * H + g0 : S * H + g1], 0.0)
                    nc.gpsimd.memset(Al16[:, S * H + g0 : S * H + g1], 0.0)

                sc_s = nc.named_scope("S")
                sc_s.__enter__()
                for S in range(TK):
                    base = 512 * (S // 8)
                    for cs, ce in _chunks(lo_of(S), H):
                        w = ce - cs
                        ps = pp.tile([P, 512], FP, tag="ps", name=f"pss{S}_{cs}")
                        # 12 score matmuls + mask matmuls (mask^T @ I adds the
                        # causal -448 pattern inside the same psum chain)
                        mm = []
                        for rp, lp in TERMS:
                            kt = kTh if lp == 0 else kTl
                            qt = qTh if rp == 0 else qTl
                            for j in range(NT // 2):
                                mm.append((
                                    ps[:, :w],
                                    _pair(kt, j * 2 * T + S * P, T, P),
                                    _pair(qt, j * 2 * H + cs, H, w),
                                ))
                        for moff, mname in MASKS[S % 8]:
                            a = base + moff * P
                            if cs <= a < ce:
                                mm.insert(len(mm) - 1, (
                                    ps[:, a - cs : a - cs + P],
                                    _pair(MT[mname], 0, P, P),
                                    _pair(ident8, 0, P, P),
                                ))
                        for n, (po, lt, rt) in enumerate(mm):
                            nc.tensor.matmul(
                                po, lhsT=lt, rhs=rt,
                                start=(n == 0), stop=(n == len(mm) - 1),
                                perf_mode=DR,
                            )
                        off = S * H + cs
                        sc = scp.tile([P, 512], FP, tag="scr")
                        nc.scalar.activation(
                            sc[:, :w], ps[:, :w], AF.Exp,
                            bias=eb_sb[:, 0:1], scale=S32,
                        )
                        nc.gpsimd.tensor_copy(Ah[:, off : off + w], sc[:, :w])
                        r32 = scp.tile([P, 512], FP, tag="scr")
                        nc.vector.tensor_sub(
                            r32[:, :w], sc[:, :w], Ah[:, off : off + w]
                        )
                        nc.vector.tensor_scalar_mul(
                            Al16[:, off : off + w], r32[:, :w], 16.0
                        )
                sc_s.__exit__(None, None, None)

                sc_r = nc.named_scope("R")
                sc_r.__enter__()
                # rowsums: ones @ (Ah | Al), DR pairs over kv tiles
                first = True
                for part, At in ((0, Ah), (1, Al16)):
                    ow = ones8 if part == 0 else ones16
                    for m in range(TK // 2):
                        lo = lo_of(2 * m)
                        for cs, ce in _chunks(lo, H):
                            w = ce - cs
                            nc.tensor.matmul(
                                rs_ps[:, cs:ce],
                                lhsT=_pair(ow, 0, P, P),
                                rhs=_pair(At, m * 2 * H + cs, H, w),
                                start=first and lo == 0,
                                stop=(part == 1 and m == TK // 2 - 1 and ce == H),
                                perf_mode=DR,
                            )
                        if lo == 0:
                            first = False
                nc.vector.reciprocal(rs_sb[:], rs_ps[:])
                sc_r.__exit__(None, None, None)

                sc_o = nc.named_scope("O")
                sc_o.__enter__()
                # attv: out [ch(ot), q] — lhsT = v, rhs = A; single
                # accumulation chain over all 16 kv tiles
                for ot in range(NT):
                    for cs, ce in _chunks(0, H):
                        ps = pp.tile([P, 512], FP, tag="ps", name=f"pso{ot}_{cs}")
                        mms = []
                        for At, vt in ((Ah, vh), (Al16, v16), (Ah, vl)):
                            for m in range(TK // 2):
                                lo = max(cs, lo_of(2 * m))
                                if lo >= ce:
                                    continue
                                mms.append(
                                    (
                                        _pair(vt, m * 2 * C + ot * P, C, P),
                                        _pair(At, m * 2 * H + lo, H, ce - lo),
                                        lo - cs,
                                        ce - lo,
                                    )
                                )
                        # widest range first so start=True covers everything
                        mms.sort(key=lambda t: t[3], reverse=True)
                        for i, (lt, rt, o0, w) in enumerate(mms):
                            nc.tensor.matmul(
                                ps[:, o0 : o0 + w],
                                lhsT=lt,
                                rhs=rt,
                                start=(i == 0),
                                stop=(i == len(mms) - 1),
                                perf_mode=DR,
                            )
                        w = ce - cs
                        ym = scp.tile([P, 512], FP, tag="scr")
                        nc.vector.tensor_mul(ym[:, :w], ps[:, :w], rs_sb[:, cs:ce])
                        ye = yep.tile([P, 512], BF, tag="ye")
                        nc.scalar.activation(
                            ye[:, :w], ym[:, :w], AF.Identity,
                            bias=be_sb[:, ot : ot + 1],
                        )
                        ci = cs // 512
                        nc.sync.dma_start(
                            yT[(ot * 2 + ci) * P : (ot * 2 + ci + 1) * P, :w],
                            ye[:, :w],
                        )
                sc_o.__exit__(None, None, None)

    _split_waits(nc)
    return nc


_NC_CACHE = None


def _get_nc():
    global _NC_CACHE
    if _NC_CACHE is None:
        _NC_CACHE = _build_nc()
    return _NC_CACHE


def _split8(a):
    """v -> (e4m3(v), e4m3(v - e4m3(v))) as fp8 arrays."""
    hi = a.astype(E4)
    lo = (a - hi.astype(np.float32)).astype(E4)
    return hi, lo


def _hl(a):
    h, l = _split8(np.ascontiguousarray(a, dtype=np.float32))
    return np.concatenate([h, l], axis=-1)


def make_in_maps(x, w_qkv, b_qkv, w_proj, b_proj):
    x = np.asarray(x, dtype=np.float32)
    w_qkv = np.asarray(w_qkv, dtype=np.float32)
    b_qkv = np.asarray(b_qkv, dtype=np.float32)
    w_proj = np.asarray(w_proj, dtype=np.float32)
    b_proj = np.asarray(b_proj, dtype=np.float32)

    wq, wk, wv = w_qkv[:C], w_qkv[C : 2 * C], w_qkv[2 * C :]
    bq, bk, bv = b_qkv[:C], b_qkv[C : 2 * C], b_qkv[2 * C :]
    wt = w_proj @ wv                       # folded V*proj weight
    beff = b_proj + w_proj @ bv

    def pack_stat(w):
        # [p, ot*1024 + ct*128 + o] = 32*w[ot*128+o, ct*128+p]
        w4 = (32.0 * w).reshape(NT, P, NT, P)       # [ot, o, ct, p]
        return w4.transpose(3, 0, 2, 1).reshape(P, NT * C)

    def pack_mov(w):
        # [p, ct*1024 + ch] = 32*w[ch, ct*128+p]
        w3 = (32.0 * w).reshape(C, NT, P)           # [ch, ct, p]
        return w3.transpose(2, 1, 0).reshape(P, NT * C)

    def pack_x(xr):
        # [p, ct*Tr + t] = xr[t, ct*128+p]
        Tr = xr.shape[0]
        x3 = xr.T.reshape(NT, P, Tr)                # [ct, p, t]
        return x3.transpose(1, 0, 2).reshape(P, NT * Tr)

    wkp = _hl(pack_stat(wk))
    wqp = _hl(pack_stat(wq))
    wvp = _hl(pack_mov(wt))
    bkp = np.ascontiguousarray(bk.reshape(NT, P).T)
    bqp = np.ascontiguousarray(bq.reshape(NT, P).T)
    bep = np.ascontiguousarray(beff.reshape(NT, P).T)

    ones = np.ones((P, 2 * P), dtype=np.float32).astype(E4)
    ones16 = np.full((P, 2 * P), 1.0 / 16.0, dtype=np.float32).astype(E4)
    # mask tiles ship TRANSPOSED (lhsT of mask^T @ I), duplicated [m|m]
    # so both DoubleRow slices add the pattern: effective bias 2*(-240)
    M8 = -240.0
    tril = np.tril(np.ones((P, P), dtype=np.float32))
    trilmT = np.where(tril > 0, 0.0, M8).astype(np.float32)
    trilmT = np.concatenate([trilmT, trilmT], axis=1).astype(E4)
    zeros = np.zeros((P, 2 * P), dtype=np.float32).astype(E4)
    negs = np.full((P, 2 * P), M8, dtype=np.float32).astype(E4)
    ident = np.concatenate(
        [np.eye(P, dtype=np.float32), np.eye(P, dtype=np.float32)], axis=1
    ).astype(E4)

    shared = dict(
        wkd=wkp, wqd=wqp, wvd=wvp, bkd=bkp, bqd=bqp, bed=bep, onesd=ones,
        ones16d=ones16, identd=ident,
        ebd=np.concatenate(
            [np.full((P, 1), ELN32, np.float32), np.zeros((P, 1), np.float32)],
            axis=1,
        ),
    )
    in_maps = []
    for core in range(8):
        b, h = core // 2, core % 2
        xb = x[b]
        qrows = np.concatenate(
            [xb[(2 * bg + h) * 256 : (2 * bg + h + 1) * 256] for bg in range(4)],
            axis=0,
        )
        in_maps.append(
            dict(
                shared,
                xd=_hl(pack_x(xb)),
                xqd=_hl(pack_x(qrows)),
                m1dd=trilmT if h == 0 else zeros,
                m1fd=negs if h == 0 else zeros,
                m2dd=negs if h == 0 else trilmT,
            )
        )
    return in_maps


def assemble_output(results):
    B = 4
    y = np.empty((B, T, C), dtype=np.float32)
    for core in range(8):
        b, h = core // 2, core % 2
        yt = np.asarray(results[core]["yT"], dtype=np.float32)
        yt = yt.reshape(NT, 2, P, 512)
        full = yt.transpose(1, 3, 0, 2).reshape(H, C)   # [q-col, ch]
        for bg in range(4):
            g = 2 * bg + h
            y[b, g * 256 : (g + 1) * 256, :] = full[bg * 256 : (bg + 1) * 256]
    return y


def kernel(x, w_qkv, b_qkv, w_proj, b_proj):
    from concourse.bass_utils import run_bass_kernel_spmd

    nc = _get_nc()
    in_maps = make_in_maps(x, w_qkv, b_qkv, w_proj, b_proj)
    res = run_bass_kernel_spmd(nc, in_maps, list(range(8)))
    return assemble_output(res.results)
